# revision 33
# baseline (speedup 1.0000x reference)
"""Trainium2 Bass kernel for LoRALayer: out = 2.0 * (x @ B) @ A.

x: [4, 4096, 4096] f32; A: [8, 4096] f32; B: [4096, 8] f32.
Sharding: data-parallel on the 16384 tokens across 8 cores (2048 each);
A/B replicated. Host-side prep (part of sharding): each core's x-shard is
shipped transposed (contraction dim on SBUF partitions) as plain bf16;
B and 2*A likewise. All-bf16 numerics land at ~7e-3 absmax-rel vs the
f32 reference (f32 PSUM accumulation), inside the 2e-2 gate. Output is
written bf16 and upcast to f32 on the host.

The PE is the bottleneck on this part (a power governor holds the PE near
1.2 GHz under sustained load; 2.4 GHz comes in credit-limited bursts):
mm1 ingests x at 256 B/cycle and mm2 emits out at 128 elem/cycle ->
131072 PE cycles/core. DMA (33.6 MB/core, ~330 B/ns/queue) hides under
it. The schedule keeps the PE stream dense:
  - token blocks [512, 512, 512, 256, 256]: big blocks minimize matmul
    count (per-instruction switch overhead ~140 ns), the split last block
    halves the un-interleavable mm2-only tail;
  - first block's input DMAs are fine-grained (256 KB) so mm1 starts ASAP;
  - mm2 of block b-1 is spread evenly among mm1 chunks of block b;
  - PSUM: 1 bank accumulates y, 7 banks round-robin mm2 output so the PE
    never waits on the PSUM->SBUF copy round-trip (~1.3 us);
  - copies alternate DVE/ACT; mid-kernel output DMAs are full 1 MB rows
    on the scalar ring; the final block's go as 256 KB quarters on the
    (by then idle) sync ring right after each copy pair.
"""

import numpy as np

P = 128
F_IN = 4096
F_OUT = 4096
RANK = 8
N_CORES = 8
SCALING = 2.0
BLOCKS = (512, 512, 512, 384, 128)   # token blocks; small final blocks keep
CGRPS = (2, 8, 8, 8, 8)              # the un-interleavable mm2 tail short
NW = 512                             # mm2 rhs width (ISA max matmul out)

_CACHE = {}


def _build_nc(T, F_in, F_out, R):
    """Build the single-core Bass program for a T-token shard."""
    from contextlib import ExitStack

    import concourse.mybir as mybir
    import concourse.tile as tile
    from concourse import bacc

    assert sum(BLOCKS) == T
    f32 = mybir.dt.float32
    bf16 = mybir.dt.bfloat16
    CH = F_in // P          # feature chunks (32)
    NB = len(BLOCKS)
    NS = F_out // NW        # output column chunks per subtile (4)

    nc = bacc.Bacc("TRN2", target_bir_lowering=False, debug=False)

    # One dram tensor per (cgrp, tblk) shape class; each sub-DMA reads one
    # contiguous [P, cgrp*tblk] slab (partition stride = slab width).
    nslab = {}
    for tb, cg in zip(BLOCKS, CGRPS):
        nslab[(cg, tb)] = nslab.get((cg, tb), 0) + CH // cg
    xt_ds = {
        k: nc.dram_tensor(
            f"xt_{k[0]}_{k[1]}", [n, P, k[0] * k[1]], bf16, kind="ExternalInput"
        ).ap()
        for k, n in nslab.items()
    }
    bpk_d = nc.dram_tensor("Bpk", [P, CH * R], bf16, kind="ExternalInput").ap()
    a2_d = nc.dram_tensor("A2", [R, F_out], bf16, kind="ExternalInput").ap()
    out_d = nc.dram_tensor("out", [T, F_out], bf16, kind="ExternalOutput").ap()

    with tile.TileContext(nc) as tc, ExitStack() as ctx:
        cpool = ctx.enter_context(tc.tile_pool(name="const", bufs=1))
        xpools = {
            (2, 512): ctx.enter_context(tc.tile_pool(name="xt0", bufs=16)),
            (8, 512): ctx.enter_context(tc.tile_pool(name="xtA", bufs=8)),
            (8, 384): ctx.enter_context(tc.tile_pool(name="xtB", bufs=4)),
            (8, 128): ctx.enter_context(tc.tile_pool(name="xtC", bufs=4)),
        }
        ytpool = ctx.enter_context(tc.tile_pool(name="yt", bufs=2))
        opool = ctx.enter_context(tc.tile_pool(name="osb", bufs=6))
        y_pp = ctx.enter_context(tc.tile_pool(name="y_ps", bufs=1, space="PSUM"))
        o_pp = ctx.enter_context(tc.tile_pool(name="o_ps", bufs=7, space="PSUM"))

        bpk_sb = cpool.tile([P, CH * R], bf16, tag="bpk_sb")
        nc.sync.dma_start(bpk_sb[:], bpk_d)
        a2_sb = cpool.tile([R, F_out], bf16, tag="a2_sb")
        # a2 is first needed by mm2 of block 0 (during block 1) -- its DMA is
        # deferred below so block 0's x stream starts sooner.

        blk_state = {}

        def emit_mm2(blk, idx):
            """idx in [0, nsub*NS): (sub, n) pair for block blk."""
            sub, n = divmod(idx, NS)
            y_sb, o_sbs, tok0, nsub = blk_state[blk]
            if n == 0:
                o_sbs[sub] = opool.tile(
                    [P, F_out], bf16, tag="o_sb", name=f"o_sb_{blk}_{sub}"
                )
            o_sb = o_sbs[sub]
            o_ps = o_pp.tile([P, NW], f32, tag="o_ps", name=f"o_ps_{blk}_{sub}_{n}")
            nc.tensor.matmul(
                o_ps[:],
                y_sb[:, sub * P:(sub + 1) * P],
                a2_sb[:, n * NW:(n + 1) * NW],
                start=True,
                stop=True,
            )
            dst = o_sb[:, n * NW:(n + 1) * NW]
            if n % 2 == 0:
                nc.vector.tensor_copy(dst, o_ps[:])
            else:
                nc.scalar.copy(dst, o_ps[:])
            trow = tok0 + sub * P
            if blk < NB - 1:
                # Mid-kernel: one full-row DMA per subtile (8 KB lines, best
                # queue efficiency; opool depth decouples PE from drain).
                if n == NS - 1:
                    nc.scalar.dma_start(out_d[trow:trow + P, :], o_sb[:])
            else:
                # Last block: quarter-row DMAs as soon as each copy pair
                # lands, issued from the (now idle) sync ring so the final
                # drain overlaps the remaining matmuls.
                if n % 2 == 1:
                    cols = slice((n - 1) * NW, (n + 1) * NW)
                    nc.sync.dma_start(out_d[trow:trow + P, cols], o_sb[:, cols])

        tok0 = 0
        slab_idx = {k: 0 for k in nslab}
        for blk in range(NB + 1):
            xts = []
            if blk < NB:
                tblk = BLOCKS[blk]
                cgrp = CGRPS[blk]
                for s in range(CH // cgrp):
                    xt_sb = xpools[(cgrp, tblk)].tile(
                        [P, cgrp, tblk], bf16, tag=f"xt_{cgrp}_{tblk}"
                    )
                    si = slab_idx[(cgrp, tblk)]
                    slab_idx[(cgrp, tblk)] = si + 1
                    nc.sync.dma_start(
                        xt_sb[:].rearrange("p c t -> p (c t)"),
                        xt_ds[(cgrp, tblk)][si],
                    )
                    xts.append(xt_sb)
                if blk == 0:
                    nc.sync.dma_start(a2_sb[:], a2_d)
                ps_y = y_pp.tile([R, max(BLOCKS)], f32, tag="ps_y")

            # Spread mm2 of the previous block evenly among this block's mm1
            # chunks (mm2 kept slightly ahead: its copy chain has latency).
            a = CH if blk < NB else 0
            b = blk_state[blk - 1][3] * NS if blk > 0 else 0
            i = j = 0
            while i < a or j < b:
                if j < b and (i >= a or j * a <= i * b):
                    emit_mm2(blk - 1, j)
                    j += 1
                else:
                    c = i
                    nc.tensor.matmul(
                        ps_y[:, :tblk],
                        bpk_sb[:, c * R:(c + 1) * R],
                        xts[c // cgrp][:, c % cgrp, :],
                        start=(c == 0),
                        stop=(c == CH - 1),
                    )
                    i += 1
            if blk > 0:
                del blk_state[blk - 1]
            if blk < NB:
                y_sb = ytpool.tile([R, max(BLOCKS)], bf16, tag="y_sb")
                nc.vector.tensor_copy(y_sb[:, :tblk], ps_y[:, :tblk])
                blk_state[blk] = (y_sb, {}, tok0, tblk // P)
                tok0 += tblk

    nc.compile()
    return nc


def _pack_inputs(x2d, A, B, T_shard, F_in, R):
    """Shard x on tokens (transposed, bf16); replicate bf16 B/2A packs."""
    import ml_dtypes

    bf16 = ml_dtypes.bfloat16
    CH = F_in // P

    # chunk-major B pack: col block c holds B chunk c ([128, R])
    bpk = np.ascontiguousarray(
        B.astype(np.float32).astype(bf16).reshape(CH, P, R)
        .transpose(1, 0, 2).reshape(P, CH * R)
    )
    a2 = np.ascontiguousarray((SCALING * A).astype(np.float32).astype(bf16))

    n_shards = x2d.shape[0] // T_shard
    in_maps = []
    for c in range(n_shards):
        xt = x2d[c * T_shard:(c + 1) * T_shard].T.astype(bf16)  # [F_in, T]
        slabs = {}
        tok0 = 0
        for tblk, cgrp in zip(BLOCKS, CGRPS):
            ndma = CH // cgrp
            blkx = xt[:, tok0:tok0 + tblk]             # [F_in, tblk]
            a4 = blkx.reshape(ndma, cgrp, P, tblk)
            a4 = a4.transpose(0, 2, 1, 3)              # [ndma, P, cgrp, tblk]
            slabs.setdefault((cgrp, tblk), []).append(
                a4.reshape(ndma, P, cgrp * tblk)
            )
            tok0 += tblk
        m = {f"xt_{k[0]}_{k[1]}": np.ascontiguousarray(np.concatenate(v))
             for k, v in slabs.items()}
        m["Bpk"] = bpk
        m["A2"] = a2
        in_maps.append(m)
    return in_maps


def kernel(x, A, B):
    from concourse.bass_utils import run_bass_kernel_spmd

    x = np.asarray(x, dtype=np.float32)
    A = np.asarray(A, dtype=np.float32)
    B = np.asarray(B, dtype=np.float32)
    orig_shape = x.shape
    x2d = x.reshape(-1, F_IN)
    T_shard = x2d.shape[0] // N_CORES

    key = (T_shard, F_IN, F_OUT, RANK)
    if key not in _CACHE:
        _CACHE[key] = _build_nc(T_shard, F_IN, F_OUT, RANK)
    nc = _CACHE[key]

    in_maps = _pack_inputs(x2d, A, B, T_shard, F_IN, RANK)
    res = run_bass_kernel_spmd(nc, in_maps, core_ids=list(range(N_CORES)))
    out = np.concatenate(
        [np.asarray(r["out"], dtype=np.float32) for r in res.results], axis=0
    )
    return out.reshape(*orig_shape[:-1], F_OUT)
